# revision 6
# baseline (speedup 1.0000x reference)
"""Trainium2 Bass kernel for nn_Attention_49907519980190.

Reference computation (b=2, n=2048, dim=1024, h=16, d=64):
    q = (x @ w_q)   -> (b, h, n, d)
    k, v = split(x @ w_vk)
    dots = (q @ k^T) * sqrt(d)          # NOTE: multiplies by 8
    attn = softmax(dots)
    out = (attn @ v) reassembled -> x @ ... @ w_out

Sharding (8 cores): batch x head-group parallel. Core c handles batch
b = c // 4 and heads 4*(c % 4) .. 4*(c % 4) + 4. Column-parallel
q/k/v projections, row-parallel out projection; the host sums the four
partial outputs per batch (the "all-reduce" of row-parallel TP).

Numerics: the softmax logits have std ~75 and the softmax is ~97%
one-hot, so Q/K/dots need fp32-class precision. The PE's in-matmul
accumulator is block-aligned (drops addends ~2^-11 below the running
max) but PSUM accumulation BETWEEN matmuls is exact fp32. So Q, K and
dots use a bf16 hi/lo decomposition: x = hi + lo with both bf16;
a@b ~= ah@bh + (al@bh + ah@bl) as separate accumulating matmuls
(measured 4.5e-6 matmul rel err vs 2.4e-3 plain bf16). V, attn@V and
the out projection are plain bf16 (errors there average out).
"""

import numpy as np

import concourse.bass as bass
import concourse.mybir as mybir
import concourse.tile as tile
from concourse import bacc
from concourse.bass_utils import run_bass_kernel_spmd
from concourse.masks import make_identity

F32 = mybir.dt.float32
BF16 = mybir.dt.bfloat16
SUB = mybir.AluOpType.subtract
MAX = mybir.AluOpType.max
ADD = mybir.AluOpType.add
AX = mybir.AxisListType.X
EXP = mybir.ActivationFunctionType.Exp

P = 128      # partitions
NTOK = 2048  # tokens per core (one batch slice)
DIM = 1024   # model dim
E = 256      # per-core projection width (4 heads x 64)
NH = 4       # heads per core
D = 64       # head dim
KO = 8       # contraction chunks of 128 over DIM
TT = 16      # token tiles of 128
SCALE = 8.0  # sqrt(D); reference MULTIPLIES by it


def build_attention_nc():
    nc = bacc.Bacc("TRN2", target_bir_lowering=False, debug=False)

    x = nc.declare_dram_parameter("x", [NTOK, DIM], F32, isOutput=False)
    wq = nc.declare_dram_parameter("wq", [DIM, E], F32, isOutput=False)
    wk = nc.declare_dram_parameter("wk", [DIM, E], F32, isOutput=False)
    wv = nc.declare_dram_parameter("wv", [DIM, E], F32, isOutput=False)
    wo = nc.declare_dram_parameter("wo", [E, DIM], F32, isOutput=False)
    y = nc.declare_dram_parameter("y", [NTOK, DIM], F32, isOutput=True)

    with tile.TileContext(nc) as tc:
        with tc.tile_pool(name="persist", bufs=1) as persist:
            # x^T in hi/lo bf16: [dim_low(128), dim_hi(8), tok]
            xh = persist.tile([P, KO, NTOK], BF16)
            xl = persist.tile([P, KO, NTOK], BF16)
            # Q^T stacked per head: rows 0:64 = q_hi, 64:128 = q_lo
            QTs = persist.tile([P, NH, NTOK], BF16)
            # K^T swapped-stacked: rows 0:64 = k_lo, 64:128 = k_hi
            KTsw = persist.tile([P, NH, NTOK], BF16)
            # K^T hi alone (base partition 0, for the main dots matmul)
            KTh = persist.tile([64, NH, NTOK], BF16)
            # V natural [tok_low, tok_hi, emb], O natural likewise
            Vb = persist.tile([P, TT, E], BF16)
            Ob = persist.tile([P, TT, E], BF16)
            # O^T [emb_low, emb_hi(2), tok]
            OTb = persist.tile([P, 2, NTOK], BF16)
            # weights
            wqh = persist.tile([P, KO, E], BF16)
            wql = persist.tile([P, KO, E], BF16)
            wkh = persist.tile([P, KO, E], BF16)
            wkl = persist.tile([P, KO, E], BF16)
            wvh = persist.tile([P, KO, E], BF16)
            wob = persist.tile([P, 2, DIM], BF16)
            ident = persist.tile([P, P], BF16)
            make_identity(nc, ident)

            # ---------------- Phase A+B: load x, split hi/lo, transpose; weights
            with tc.tile_pool(name="stage", bufs=3) as stage:
                for tt in range(TT):
                    ts = slice(tt * P, (tt + 1) * P)
                    xf = stage.tile([P, DIM], F32, tag="xf")
                    nc.sync.dma_start(out=xf, in_=x[ts, :])
                    xhn = stage.tile([P, DIM], BF16, tag="xhn")
                    xln = stage.tile([P, DIM], BF16, tag="xln")
                    nc.vector.tensor_copy(out=xhn, in_=xf)
                    nc.vector.tensor_tensor(out=xln, in0=xf, in1=xhn, op=SUB)
                    nc.sync.dma_start_transpose(out=xh[:, :, ts], in_=xhn)
                    nc.sync.dma_start_transpose(out=xl[:, :, ts], in_=xln)

                for wsrc, hdst, ldst in ((wq, wqh, wql), (wk, wkh, wkl)):
                    wf = stage.tile([P, KO, E], F32, tag="wf", bufs=1)
                    nc.sync.dma_start(
                        out=wf, in_=wsrc[:, :].rearrange("(ko p) e -> p ko e", p=P)
                    )
                    nc.vector.tensor_copy(out=hdst, in_=wf)
                    nc.vector.tensor_tensor(out=ldst, in0=wf, in1=hdst, op=SUB)
                wf = stage.tile([P, KO, E], F32, tag="wf", bufs=1)
                nc.sync.dma_start(
                    out=wf, in_=wv[:, :].rearrange("(ko p) e -> p ko e", p=P)
                )
                nc.vector.tensor_copy(out=wvh, in_=wf)
                wof = stage.tile([P, 2, DIM], F32, tag="wof", bufs=1)
                nc.sync.dma_start(
                    out=wof, in_=wo[:, :].rearrange("(eo p) d -> p eo d", p=P)
                )
                nc.vector.tensor_copy(out=wob, in_=wof)

            # ---------------- Phase C: projections K^T, Q^T (hi/lo), V (plain)
            with tc.tile_pool(name="psA", bufs=2, space="PSUM") as psA:
                def proj_hilo(wh, wl, m, n):
                    """psum[e_low, tok] = (w^T @ x^T)[m-chunk, n-chunk], hi/lo."""
                    pr = psA.tile([P, 512], F32, tag="pr")
                    ms = slice(m * P, (m + 1) * P)
                    ns = slice(n * 512, (n + 1) * 512)
                    for c in range(KO):
                        nc.tensor.matmul(
                            pr[:, :], wh[:, c, ms], xh[:, c, ns],
                            start=(c == 0), stop=False,
                        )
                    for c in range(KO):
                        nc.tensor.matmul(
                            pr[:, :], wl[:, c, ms], xh[:, c, ns],
                            start=False, stop=False,
                        )
                    for c in range(KO):
                        nc.tensor.matmul(
                            pr[:, :], wh[:, c, ms], xl[:, c, ns],
                            start=False, stop=(c == KO - 1),
                        )
                    return pr

                for m in range(2):
                    for n in range(4):
                        ns = slice(n * 512, (n + 1) * 512)
                        pr = proj_hilo(wkh, wkl, m, n)
                        for hh in range(2):
                            h = 2 * m + hh
                            rows = slice(hh * 64, hh * 64 + 64)
                            nc.vector.tensor_copy(out=KTh[0:64, h, ns], in_=pr[rows, :])
                            nc.vector.tensor_copy(
                                out=KTsw[64:128, h, ns], in_=pr[rows, :]
                            )
                            nc.vector.tensor_tensor(
                                out=KTsw[0:64, h, ns], in0=pr[rows, :],
                                in1=KTh[0:64, h, ns], op=SUB,
                            )
                for m in range(2):
                    for n in range(4):
                        ns = slice(n * 512, (n + 1) * 512)
                        pr = proj_hilo(wqh, wql, m, n)
                        for hh in range(2):
                            h = 2 * m + hh
                            rows = slice(hh * 64, hh * 64 + 64)
                            nc.vector.tensor_copy(out=QTs[0:64, h, ns], in_=pr[rows, :])
                            nc.vector.tensor_tensor(
                                out=QTs[64:128, h, ns], in0=pr[rows, :],
                                in1=QTs[0:64, h, ns], op=SUB,
                            )
                for tm in range(TT):
                    ms = slice(tm * P, (tm + 1) * P)
                    pr = psA.tile([P, 512], F32, tag="pr")
                    for c in range(KO):
                        nc.tensor.matmul(
                            pr[:, :E], xh[:, c, ms], wvh[:, c, :],
                            start=(c == 0), stop=(c == KO - 1),
                        )
                    nc.vector.tensor_copy(out=Vb[:, tm, :], in_=pr[:, :E])

            # ---------------- Phase D: attention per (head, query-tile)
            with (
                tc.tile_pool(name="psS", bufs=3, space="PSUM") as psS,
                tc.tile_pool(name="psO", bufs=2, space="PSUM") as psO,
                tc.tile_pool(name="attn_sb", bufs=3) as attn_sb,
                tc.tile_pool(name="attn_small", bufs=4) as attn_small,
            ):
                for h in range(NH):
                    for it in range(TT):
                        isl = slice(it * P, (it + 1) * P)
                        halves = []
                        mx2 = attn_small.tile([P, 2], F32, tag="mx2")
                        for half in range(2):
                            Sh = psS.tile([P, 1024], F32, tag="S")
                            for n2 in range(2):
                                ns = slice(
                                    (half * 2 + n2) * 512, (half * 2 + n2 + 1) * 512
                                )
                                nc.tensor.matmul(
                                    Sh[:, n2 * 512:(n2 + 1) * 512],
                                    QTs[0:64, h, isl], KTh[0:64, h, ns],
                                    start=True, stop=False,
                                )
                                nc.tensor.matmul(
                                    Sh[:, n2 * 512:(n2 + 1) * 512],
                                    QTs[:, h, isl], KTsw[:, h, ns],
                                    start=False, stop=True,
                                )
                            nc.vector.tensor_reduce(
                                out=mx2[:, half:half + 1], in_=Sh, axis=AX, op=MAX
                            )
                            halves.append(Sh)
                        nmx = attn_small.tile([P, 1], F32, tag="nmx")
                        nc.vector.tensor_reduce(
                            out=nmx, in_=mx2, axis=AX, op=MAX, negate=True
                        )
                        bias8 = attn_small.tile([P, 1], F32, tag="bias8")
                        nc.vector.tensor_scalar_mul(bias8, nmx, SCALE)
                        den2 = attn_small.tile([P, 2], F32, tag="den2")
                        Pb = attn_sb.tile([P, NTOK], BF16, tag="Pb")
                        for half in range(2):
                            nc.scalar.activation(
                                out=Pb[:, half * 1024:(half + 1) * 1024],
                                in_=halves[half], func=EXP,
                                bias=bias8, scale=SCALE,
                                accum_out=den2[:, half:half + 1],
                            )
                        den = attn_small.tile([P, 1], F32, tag="den")
                        nc.vector.tensor_reduce(out=den, in_=den2, axis=AX, op=ADD)
                        rec = attn_small.tile([P, 1], F32, tag="rec")
                        nc.vector.reciprocal(out=rec, in_=den)
                        PT = attn_sb.tile([P, TT, P], BF16, tag="PT")
                        nc.sync.dma_start_transpose(out=PT, in_=Pb)
                        O_ps = psO.tile([P, D], F32, tag="O")
                        for jo in range(TT):
                            nc.tensor.matmul(
                                O_ps[:, :], PT[:, jo, :],
                                Vb[:, jo, h * D:(h + 1) * D],
                                start=(jo == 0), stop=(jo == TT - 1),
                            )
                        nc.vector.tensor_scalar_mul(
                            Ob[:, it, h * D:(h + 1) * D], O_ps[:, :], rec
                        )

            # ---------------- Phase E: O^T then y = O @ wo
            with (
                tc.tile_pool(name="psE", bufs=2, space="PSUM") as psE,
                tc.tile_pool(name="ysb", bufs=3) as ysb,
            ):
                for tt in range(TT):
                    ts = slice(tt * P, (tt + 1) * P)
                    for eo in range(2):
                        pt = psE.tile([P, P], BF16, tag="pt")
                        nc.tensor.transpose(
                            pt[:, :], Ob[:, tt, eo * P:(eo + 1) * P], ident[:, :]
                        )
                        nc.vector.tensor_copy(out=OTb[:, eo, ts], in_=pt)
                for tm in range(TT):
                    ms = slice(tm * P, (tm + 1) * P)
                    for n in range(2):
                        ns = slice(n * 512, (n + 1) * 512)
                        yp = psE.tile([P, 512], F32, tag="yp")
                        for eo in range(2):
                            nc.tensor.matmul(
                                yp[:, :], OTb[:, eo, ms], wob[:, eo, ns],
                                start=(eo == 0), stop=(eo == 1),
                            )
                        yo = ysb.tile([P, 512], F32, tag="yo")
                        nc.scalar.copy(out=yo, in_=yp)
                        nc.sync.dma_start(out=y[ms, ns], in_=yo)

    nc.compile()
    return nc


_NC_CACHE = None


def _get_nc():
    global _NC_CACHE
    if _NC_CACHE is None:
        _NC_CACHE = build_attention_nc()
    return _NC_CACHE


def kernel(x, w_q, w_vk, w_out, **run_kwargs):
    """Full inputs in, full output out. Shards over 8 NeuronCores."""
    b, n, dim = x.shape
    assert (b, n, dim) == (2, 2048, 1024)
    w_k = w_vk[:, :1024]
    w_v = w_vk[:, 1024:]

    in_maps = []
    for c in range(8):
        bi = c // 4
        hg = c % 4
        cs = slice(hg * E, (hg + 1) * E)
        in_maps.append({
            "x": np.ascontiguousarray(x[bi]).astype(np.float32),
            "wq": np.ascontiguousarray(w_q[:, cs]).astype(np.float32),
            "wk": np.ascontiguousarray(w_k[:, cs]).astype(np.float32),
            "wv": np.ascontiguousarray(w_v[:, cs]).astype(np.float32),
            "wo": np.ascontiguousarray(w_out[cs, :]).astype(np.float32),
        })

    nc = _get_nc()
    res = run_bass_kernel_spmd(nc, in_maps, core_ids=list(range(8)), **run_kwargs)
    out = np.zeros((2, 2048, 1024), dtype=np.float32)
    for c in range(8):
        out[c // 4] += res.results[c]["y"]
    if run_kwargs:
        kernel.last_results = res
    return out


# revision 15
# speedup vs baseline: 1.0901x; 1.0901x over previous
"""Trainium2 Bass kernel for nn_Attention_49907519980190.

Reference computation (b=2, n=2048, dim=1024, h=16, d=64):
    q = (x @ w_q)   -> (b, h, n, d)
    k, v = split(x @ w_vk)
    dots = (q @ k^T) * sqrt(d)          # NOTE: multiplies by 8
    attn = softmax(dots)
    out = (attn @ v) reassembled -> (b, n, h*d) @ w_out

Sharding (8 cores): batch x head-group parallel. Core c handles batch
b = c // 4 and heads 4*(c % 4) .. 4*(c % 4) + 4. Column-parallel
q/k/v projections, row-parallel out projection; the host sums the four
partial outputs per batch (the "all-reduce" of row-parallel TP).

Numerics: the softmax logits have std ~75 and the softmax is ~97%
one-hot, so Q/K/dots need fp32-class precision. The PE's in-matmul
accumulator is block-aligned (drops addends ~2^-11 below the running
max) but PSUM accumulation BETWEEN matmuls is exact fp32. So Q, K and
dots use a bf16 hi/lo decomposition: x = hi + lo with both bf16;
a@b ~= ah@bh + (al@bh + ah@bl) as separate accumulating matmuls
(measured 4.5e-6 matmul rel err vs 2.4e-3 plain bf16). V, attn@V and
the out projection are plain bf16 (errors there average out).

Schedule notes (from perfetto): keep the PE stream free of stalls --
software-pipeline the attention inner loop (PV for iteration N-2 is
issued between the dots of iteration N), start projections as soon as
the first token-group of x^T lands (x^T is split into 4 token-group
tiles), and spread DMA issue / PSUM eviction across sync, scalar,
vector and gpsimd so no engine's sequencer serializes the chain.
"""

import numpy as np

import concourse.bass as bass
import concourse.mybir as mybir
import concourse.tile as tile
from concourse import bacc
from concourse.bass_utils import run_bass_kernel_spmd
from concourse.masks import make_identity

F32 = mybir.dt.float32
BF16 = mybir.dt.bfloat16
SUB = mybir.AluOpType.subtract
MAX = mybir.AluOpType.max
ADD = mybir.AluOpType.add
AX = mybir.AxisListType.X
EXP = mybir.ActivationFunctionType.Exp

P = 128      # partitions
NTOK = 2048  # tokens per core (one batch slice)
DIM = 1024   # model dim
E = 256      # per-core projection width (4 heads x 64)
NH = 4       # heads per core
D = 64       # head dim
KO = 8       # contraction chunks of 128 over DIM
TT = 16      # token tiles of 128
NG = 4       # token groups (of 512) for x^T
SCALE = 8.0  # sqrt(D); reference MULTIPLIES by it


def build_attention_nc():
    nc = bacc.Bacc("TRN2", target_bir_lowering=False, debug=False)

    x = nc.declare_dram_parameter("x", [NTOK, DIM], F32, isOutput=False)
    wq = nc.declare_dram_parameter("wq", [DIM, E], F32, isOutput=False)
    wk = nc.declare_dram_parameter("wk", [DIM, E], F32, isOutput=False)
    wv = nc.declare_dram_parameter("wv", [DIM, E], F32, isOutput=False)
    wo = nc.declare_dram_parameter("wo", [E, DIM], F32, isOutput=False)
    y = nc.declare_dram_parameter("y", [NTOK, DIM], F32, isOutput=True)

    with tile.TileContext(nc) as tc:
        with tc.tile_pool(name="persist", bufs=1) as persist:
            # x^T hi/lo bf16 in 4 token-group tiles: [dim_low, dim_hi, 512 tok]
            xh = []
            xl = []
            for g in range(NG):
                xh_g = persist.tile([P, KO, 512], BF16, tag=f"xh{g}")
                xl_g = persist.tile([P, KO, 512], BF16, tag=f"xl{g}")
                xh.append(xh_g)
                xl.append(xl_g)
            # Q^T stacked per head: rows 0:64 = q_hi, 64:128 = q_lo
            QTs = persist.tile([P, NH, NTOK], BF16)
            # K^T swapped-stacked: rows 0:64 = k_lo, 64:128 = k_hi
            KTsw = persist.tile([P, NH, NTOK], BF16)
            # K^T hi alone (base partition 0, for the main dots matmul)
            KTh = persist.tile([64, NH, NTOK], BF16)
            # V natural [tok_low, tok_hi, emb], O natural likewise
            Vb = persist.tile([P, TT, E], BF16)
            Ob = persist.tile([P, TT, E], BF16)
            # O^T [emb_low, emb_hi(2), tok]
            OTb = persist.tile([P, 2, NTOK], BF16)
            # weights
            wqh = persist.tile([P, KO, E], BF16)
            wql = persist.tile([P, KO, E], BF16)
            wkh = persist.tile([P, KO, E], BF16)
            wkl = persist.tile([P, KO, E], BF16)
            wvh = persist.tile([P, KO, E], BF16)
            wob = persist.tile([P, 2, DIM], BF16)
            ident = persist.tile([P, P], BF16)
            make_identity(nc, ident)

            # ------------- Phase A/B: weights, then x (split hi/lo + transpose)
            with tc.tile_pool(name="stage", bufs=3) as stage:
                # weights first so projections can start the moment x^T lands
                for wsrc, hdst, ldst in ((wk, wkh, wkl), (wq, wqh, wql)):
                    wf = stage.tile([P, KO, E], F32, tag="wf", bufs=1)
                    nc.sync.dma_start(
                        out=wf, in_=wsrc[:, :].rearrange("(ko p) e -> p ko e", p=P)
                    )
                    nc.vector.tensor_copy(out=hdst, in_=wf)
                    nc.vector.tensor_tensor(out=ldst, in0=wf, in1=hdst, op=SUB)
                wf = stage.tile([P, KO, E], F32, tag="wf", bufs=1)
                nc.sync.dma_start(
                    out=wf, in_=wv[:, :].rearrange("(ko p) e -> p ko e", p=P)
                )
                nc.vector.tensor_copy(out=wvh, in_=wf)
                wof = stage.tile([P, 2, DIM], F32, tag="wof", bufs=1)
                nc.sync.dma_start(
                    out=wof, in_=wo[:, :].rearrange("(eo p) d -> p eo d", p=P)
                )
                nc.vector.tensor_copy(out=wob, in_=wof)

                for tt in range(TT):
                    g, lt = tt // 4, tt % 4
                    ts = slice(tt * P, (tt + 1) * P)
                    gs = slice(lt * P, (lt + 1) * P)
                    xf = stage.tile([P, DIM], F32, tag="xf")
                    nc.sync.dma_start(out=xf, in_=x[ts, :])
                    xhn = stage.tile([P, DIM], BF16, tag="xhn")
                    xln = stage.tile([P, DIM], BF16, tag="xln")
                    nc.vector.tensor_copy(out=xhn, in_=xf)
                    nc.vector.tensor_tensor(out=xln, in0=xf, in1=xhn, op=SUB)
                    nc.sync.dma_start_transpose(out=xh[g][:, :, gs], in_=xhn)
                    nc.sync.dma_start_transpose(out=xl[g][:, :, gs], in_=xln)

            # ------------- Phase C: projections K^T, Q^T (hi/lo), V (plain)
            with tc.tile_pool(name="psA", bufs=2, space="PSUM") as psA:
                def proj_hilo(wh, wl, m, g):
                    pr = psA.tile([P, 512], F32, tag="pr")
                    ms = slice(m * P, (m + 1) * P)
                    for c in range(KO):
                        nc.tensor.matmul(
                            pr[:, :], wh[:, c, ms], xh[g][:, c, :],
                            start=(c == 0), stop=False,
                        )
                    for c in range(KO):
                        nc.tensor.matmul(
                            pr[:, :], wl[:, c, ms], xh[g][:, c, :],
                            start=False, stop=False,
                        )
                    for c in range(KO):
                        nc.tensor.matmul(
                            pr[:, :], wh[:, c, ms], xl[g][:, c, :],
                            start=False, stop=(c == KO - 1),
                        )
                    return pr

                for g in range(NG):
                    ns = slice(g * 512, (g + 1) * 512)
                    for m in range(2):
                        pr = proj_hilo(wkh, wkl, m, g)
                        for hh in range(2):
                            h = 2 * m + hh
                            rows = slice(hh * 64, hh * 64 + 64)
                            nc.scalar.copy(out=KTh[0:64, h, ns], in_=pr[rows, :])
                            nc.scalar.copy(out=KTsw[64:128, h, ns], in_=pr[rows, :])
                            nc.vector.tensor_tensor(
                                out=KTsw[0:64, h, ns], in0=pr[rows, :],
                                in1=KTh[0:64, h, ns], op=SUB,
                            )
                    for m in range(2):
                        pr = proj_hilo(wqh, wql, m, g)
                        for hh in range(2):
                            h = 2 * m + hh
                            rows = slice(hh * 64, hh * 64 + 64)
                            nc.scalar.copy(out=QTs[0:64, h, ns], in_=pr[rows, :])
                            nc.vector.tensor_tensor(
                                out=QTs[64:128, h, ns], in0=pr[rows, :],
                                in1=QTs[0:64, h, ns], op=SUB,
                            )
                    for lt in range(4):
                        tm = g * 4 + lt
                        pr = psA.tile([P, 512], F32, tag="pr")
                        for c in range(KO):
                            nc.tensor.matmul(
                                pr[:, :E], xh[g][:, c, lt * P:(lt + 1) * P],
                                wvh[:, c, :],
                                start=(c == 0), stop=(c == KO - 1),
                            )
                        nc.scalar.copy(out=Vb[:, tm, :], in_=pr[:, :E])

            # ------------- Phase D: attention, software-pipelined depth 2
            with (
                tc.tile_pool(name="psS", bufs=3, space="PSUM") as psS,
                tc.tile_pool(name="psO", bufs=2, space="PSUM") as psO,
                tc.tile_pool(name="attn_sb", bufs=3) as attn_sb,
                tc.tile_pool(name="attn_small", bufs=4) as attn_small,
            ):
                pending = []

                def issue_dots(h, it):
                    """dots for (h, it): 2 stationaries, 8 matmuls, 2 S halves."""
                    isl = slice(it * P, (it + 1) * P)
                    S0 = psS.tile([P, 1024], F32, tag="S")
                    S1 = psS.tile([P, 1024], F32, tag="S")
                    halves = [S0, S1]
                    HOIST = True
                    if HOIST:
                        for nn in range(4):
                            ns = slice(nn * 512, (nn + 1) * 512)
                            Sh = halves[nn // 2]
                            co = (nn % 2) * 512
                            nc.tensor.matmul(
                                Sh[:, co:co + 512], QTs[0:64, h, isl], KTh[0:64, h, ns],
                                start=True, stop=False,
                            )
                        for nn in range(4):
                            ns = slice(nn * 512, (nn + 1) * 512)
                            Sh = halves[nn // 2]
                            co = (nn % 2) * 512
                            nc.tensor.matmul(
                                Sh[:, co:co + 512], QTs[:, h, isl], KTsw[:, h, ns],
                                start=False, stop=True,
                            )
                    else:
                        for nn in range(4):
                            ns = slice(nn * 512, (nn + 1) * 512)
                            Sh = halves[nn // 2]
                            co = (nn % 2) * 512
                            nc.tensor.matmul(
                                Sh[:, co:co + 512], QTs[0:64, h, isl], KTh[0:64, h, ns],
                                start=True, stop=False,
                            )
                            nc.tensor.matmul(
                                Sh[:, co:co + 512], QTs[:, h, isl], KTsw[:, h, ns],
                                start=False, stop=True,
                            )
                    return halves

                def issue_softmax(h, it, halves):
                    mx2 = attn_small.tile([P, 2], F32, tag="mx2")
                    for half in range(2):
                        nc.vector.tensor_reduce(
                            out=mx2[:, half:half + 1], in_=halves[half],
                            axis=AX, op=MAX,
                        )
                    nmx = attn_small.tile([P, 1], F32, tag="nmx")
                    nc.vector.tensor_reduce(
                        out=nmx, in_=mx2, axis=AX, op=MAX, negate=True
                    )
                    bias8 = attn_small.tile([P, 1], F32, tag="bias8")
                    nc.vector.tensor_scalar_mul(bias8, nmx, SCALE)
                    den2 = attn_small.tile([P, 2], F32, tag="den2")
                    Pb = attn_sb.tile([P, NTOK], BF16, tag="Pb")
                    for half in range(2):
                        nc.scalar.activation(
                            out=Pb[:, half * 1024:(half + 1) * 1024],
                            in_=halves[half], func=EXP,
                            bias=bias8, scale=SCALE,
                            accum_out=den2[:, half:half + 1],
                        )
                    den = attn_small.tile([P, 1], F32, tag="den")
                    nc.vector.tensor_reduce(out=den, in_=den2, axis=AX, op=ADD)
                    rec = attn_small.tile([P, 1], F32, tag="rec")
                    nc.vector.reciprocal(out=rec, in_=den)
                    PT = attn_sb.tile([P, TT, P], BF16, tag="PT")
                    nc.sync.dma_start_transpose(out=PT, in_=Pb)
                    return PT, rec

                def issue_pv(h, it, PT, rec):
                    O_ps = psO.tile([P, D], F32, tag="O")
                    for jo in range(TT):
                        nc.tensor.matmul(
                            O_ps[:, :], PT[:, jo, :], Vb[:, jo, h * D:(h + 1) * D],
                            start=(jo == 0), stop=(jo == TT - 1),
                        )
                    nc.vector.tensor_scalar_mul(
                        Ob[:, it, h * D:(h + 1) * D], O_ps[:, :], rec
                    )

                PIPELINE = True
                for h in range(NH):
                    for it in range(TT):
                        halves = issue_dots(h, it)
                        if PIPELINE and len(pending) >= 2:
                            issue_pv(*pending.pop(0))
                        PT, rec = issue_softmax(h, it, halves)
                        pending.append((h, it, PT, rec))
                        if not PIPELINE:
                            issue_pv(*pending.pop(0))
                while pending:
                    issue_pv(*pending.pop(0))

            # ------------- Phase E: O^T then y = O @ wo
            with (
                tc.tile_pool(name="psE", bufs=2, space="PSUM") as psE,
                tc.tile_pool(name="ysb", bufs=3) as ysb,
            ):
                for tt in range(TT):
                    ts = slice(tt * P, (tt + 1) * P)
                    for eo in range(2):
                        pt = psE.tile([P, P], BF16, tag="pt")
                        nc.tensor.transpose(
                            pt[:, :], Ob[:, tt, eo * P:(eo + 1) * P], ident[:, :]
                        )
                        nc.vector.tensor_copy(out=OTb[:, eo, ts], in_=pt)
                for tm in range(TT):
                    ms = slice(tm * P, (tm + 1) * P)
                    for n in range(2):
                        ns = slice(n * 512, (n + 1) * 512)
                        yp = psE.tile([P, 512], F32, tag="yp")
                        for eo in range(2):
                            nc.tensor.matmul(
                                yp[:, :], OTb[:, eo, ms], wob[:, eo, ns],
                                start=(eo == 0), stop=(eo == 1),
                            )
                        yo = ysb.tile([P, 512], F32, tag="yo")
                        nc.scalar.copy(out=yo, in_=yp)
                        nc.sync.dma_start(out=y[ms, ns], in_=yo)

    nc.compile()
    return nc


_NC_CACHE = None


def _get_nc():
    global _NC_CACHE
    if _NC_CACHE is None:
        _NC_CACHE = build_attention_nc()
    return _NC_CACHE


def kernel(x, w_q, w_vk, w_out, **run_kwargs):
    """Full inputs in, full output out. Shards over 8 NeuronCores."""
    b, n, dim = x.shape
    assert (b, n, dim) == (2, 2048, 1024)
    w_k = w_vk[:, :1024]
    w_v = w_vk[:, 1024:]

    in_maps = []
    for c in range(8):
        bi = c // 4
        hg = c % 4
        cs = slice(hg * E, (hg + 1) * E)
        in_maps.append({
            "x": np.ascontiguousarray(x[bi]).astype(np.float32),
            "wq": np.ascontiguousarray(w_q[:, cs]).astype(np.float32),
            "wk": np.ascontiguousarray(w_k[:, cs]).astype(np.float32),
            "wv": np.ascontiguousarray(w_v[:, cs]).astype(np.float32),
            "wo": np.ascontiguousarray(w_out[cs, :]).astype(np.float32),
        })

    nc = _get_nc()
    res = run_bass_kernel_spmd(nc, in_maps, core_ids=list(range(8)), **run_kwargs)
    out = np.zeros((2, 2048, 1024), dtype=np.float32)
    for c in range(8):
        out[c // 4] += res.results[c]["y"]
    if run_kwargs:
        kernel.last_results = res
    return out


# revision 16
# speedup vs baseline: 1.0927x; 1.0023x over previous
"""Trainium2 Bass kernel for nn_Attention_49907519980190.

Reference computation (b=2, n=2048, dim=1024, h=16, d=64):
    q = (x @ w_q)   -> (b, h, n, d)
    k, v = split(x @ w_vk)
    dots = (q @ k^T) * sqrt(d)          # NOTE: multiplies by 8
    attn = softmax(dots)
    out = (attn @ v) reassembled -> (b, n, h*d) @ w_out

Sharding (8 cores): batch x head-group parallel. Core c handles batch
b = c // 4 and heads 4*(c % 4) .. 4*(c % 4) + 4. Column-parallel
q/k/v projections, row-parallel out projection; the host sums the four
partial outputs per batch (the "all-reduce" of row-parallel TP).

Numerics: the softmax logits have std ~75 and the softmax is ~97%
one-hot, so Q/K/dots need fp32-class precision. The PE's in-matmul
accumulator is block-aligned (drops addends ~2^-11 below the running
max) but PSUM accumulation BETWEEN matmuls is exact fp32. So Q, K and
dots use a bf16 hi/lo decomposition: x = hi + lo with both bf16;
a@b ~= ah@bh + (al@bh + ah@bl) as separate accumulating matmuls
(measured 4.5e-6 matmul rel err vs 2.4e-3 plain bf16). V, attn@V and
the out projection are plain bf16 (errors there average out).

Schedule notes (from perfetto): keep the PE stream free of stalls --
software-pipeline the attention inner loop (PV for iteration N-2 is
issued between the dots of iteration N), start projections as soon as
the first token-group of x^T lands (x^T is split into 4 token-group
tiles), and spread DMA issue / PSUM eviction across sync, scalar,
vector and gpsimd so no engine's sequencer serializes the chain.
"""

import numpy as np

import concourse.bass as bass
import concourse.mybir as mybir
import concourse.tile as tile
from concourse import bacc
from concourse.bass_utils import run_bass_kernel_spmd
from concourse.masks import make_identity

F32 = mybir.dt.float32
BF16 = mybir.dt.bfloat16
SUB = mybir.AluOpType.subtract
MAX = mybir.AluOpType.max
ADD = mybir.AluOpType.add
AX = mybir.AxisListType.X
EXP = mybir.ActivationFunctionType.Exp

P = 128      # partitions
NTOK = 2048  # tokens per core (one batch slice)
DIM = 1024   # model dim
E = 256      # per-core projection width (4 heads x 64)
NH = 4       # heads per core
D = 64       # head dim
KO = 8       # contraction chunks of 128 over DIM
TT = 16      # token tiles of 128
NG = 4       # token groups (of 512) for x^T
SCALE = 8.0  # sqrt(D); reference MULTIPLIES by it


def build_attention_nc():
    nc = bacc.Bacc("TRN2", target_bir_lowering=False, debug=False)

    x = nc.declare_dram_parameter("x", [NTOK, DIM], F32, isOutput=False)
    wq = nc.declare_dram_parameter("wq", [DIM, E], F32, isOutput=False)
    wk = nc.declare_dram_parameter("wk", [DIM, E], F32, isOutput=False)
    wv = nc.declare_dram_parameter("wv", [DIM, E], F32, isOutput=False)
    wo = nc.declare_dram_parameter("wo", [E, DIM], F32, isOutput=False)
    y = nc.declare_dram_parameter("y", [NTOK, DIM], F32, isOutput=True)

    with tile.TileContext(nc) as tc:
        with tc.tile_pool(name="persist", bufs=1) as persist:
            # x^T hi/lo bf16 in 4 token-group tiles: [dim_low, dim_hi, 512 tok]
            xh = []
            xl = []
            for g in range(NG):
                xh_g = persist.tile([P, KO, 512], BF16, tag=f"xh{g}")
                xl_g = persist.tile([P, KO, 512], BF16, tag=f"xl{g}")
                xh.append(xh_g)
                xl.append(xl_g)
            # Q^T stacked per head: rows 0:64 = q_hi, 64:128 = q_lo
            QTs = persist.tile([P, NH, NTOK], BF16)
            # K^T swapped-stacked: rows 0:64 = k_lo, 64:128 = k_hi
            KTsw = persist.tile([P, NH, NTOK], BF16)
            # K^T hi alone (base partition 0, for the main dots matmul)
            KTh = persist.tile([64, NH, NTOK], BF16)
            # V natural [tok_low, tok_hi, emb], O natural likewise
            Vb = persist.tile([P, TT, E], BF16)
            Ob = persist.tile([P, TT, E], BF16)
            # O^T [emb_low, emb_hi(2), tok]
            OTb = persist.tile([P, 2, NTOK], BF16)
            # weights
            wqh = persist.tile([P, KO, E], BF16)
            wql = persist.tile([P, KO, E], BF16)
            wkh = persist.tile([P, KO, E], BF16)
            wkl = persist.tile([P, KO, E], BF16)
            wvh = persist.tile([P, KO, E], BF16)
            wob = persist.tile([P, 2, DIM], BF16)
            ident = persist.tile([P, P], BF16)
            make_identity(nc, ident)

            # ------------- Phase A/B: weights, then x (split hi/lo + transpose)
            with tc.tile_pool(name="stage", bufs=3) as stage:
                # weights first so projections can start the moment x^T lands
                for wsrc, hdst, ldst in ((wk, wkh, wkl), (wq, wqh, wql)):
                    wf = stage.tile([P, KO, E], F32, tag="wf", bufs=1)
                    nc.sync.dma_start(
                        out=wf, in_=wsrc[:, :].rearrange("(ko p) e -> p ko e", p=P)
                    )
                    nc.vector.tensor_copy(out=hdst, in_=wf)
                    nc.vector.tensor_tensor(out=ldst, in0=wf, in1=hdst, op=SUB)
                wf = stage.tile([P, KO, E], F32, tag="wf", bufs=1)
                nc.sync.dma_start(
                    out=wf, in_=wv[:, :].rearrange("(ko p) e -> p ko e", p=P)
                )
                nc.vector.tensor_copy(out=wvh, in_=wf)
                wof = stage.tile([P, 2, DIM], F32, tag="wof", bufs=1)
                nc.sync.dma_start(
                    out=wof, in_=wo[:, :].rearrange("(eo p) d -> p eo d", p=P)
                )
                nc.vector.tensor_copy(out=wob, in_=wof)

                for tt in range(TT):
                    g, lt = tt // 4, tt % 4
                    ts = slice(tt * P, (tt + 1) * P)
                    gs = slice(lt * P, (lt + 1) * P)
                    xf = stage.tile([P, DIM], F32, tag="xf")
                    nc.sync.dma_start(out=xf, in_=x[ts, :])
                    xhn = stage.tile([P, DIM], BF16, tag="xhn")
                    xln = stage.tile([P, DIM], BF16, tag="xln")
                    nc.vector.tensor_copy(out=xhn, in_=xf)
                    nc.vector.tensor_tensor(out=xln, in0=xf, in1=xhn, op=SUB)
                    nc.sync.dma_start_transpose(out=xh[g][:, :, gs], in_=xhn)
                    nc.sync.dma_start_transpose(out=xl[g][:, :, gs], in_=xln)

            # ------------- Phase C: projections K^T, Q^T (hi/lo), V (plain)
            with tc.tile_pool(name="psA", bufs=2, space="PSUM") as psA:
                def proj_hilo(wh, wl, m, g):
                    pr = psA.tile([P, 512], F32, tag="pr")
                    ms = slice(m * P, (m + 1) * P)
                    for c in range(KO):
                        nc.tensor.matmul(
                            pr[:, :], wh[:, c, ms], xh[g][:, c, :],
                            start=(c == 0), stop=False,
                        )
                    for c in range(KO):
                        nc.tensor.matmul(
                            pr[:, :], wl[:, c, ms], xh[g][:, c, :],
                            start=False, stop=False,
                        )
                    for c in range(KO):
                        nc.tensor.matmul(
                            pr[:, :], wh[:, c, ms], xl[g][:, c, :],
                            start=False, stop=(c == KO - 1),
                        )
                    return pr

                for g in range(NG):
                    ns = slice(g * 512, (g + 1) * 512)
                    for m in range(2):
                        pr = proj_hilo(wkh, wkl, m, g)
                        for hh in range(2):
                            h = 2 * m + hh
                            rows = slice(hh * 64, hh * 64 + 64)
                            nc.scalar.copy(out=KTh[0:64, h, ns], in_=pr[rows, :])
                            nc.scalar.copy(out=KTsw[64:128, h, ns], in_=pr[rows, :])
                            nc.vector.tensor_tensor(
                                out=KTsw[0:64, h, ns], in0=pr[rows, :],
                                in1=KTh[0:64, h, ns], op=SUB,
                            )
                    for m in range(2):
                        pr = proj_hilo(wqh, wql, m, g)
                        for hh in range(2):
                            h = 2 * m + hh
                            rows = slice(hh * 64, hh * 64 + 64)
                            nc.scalar.copy(out=QTs[0:64, h, ns], in_=pr[rows, :])
                            nc.vector.tensor_tensor(
                                out=QTs[64:128, h, ns], in0=pr[rows, :],
                                in1=QTs[0:64, h, ns], op=SUB,
                            )
                    for lt in range(4):
                        tm = g * 4 + lt
                        pr = psA.tile([P, 512], F32, tag="pr")
                        for c in range(KO):
                            nc.tensor.matmul(
                                pr[:, :E], xh[g][:, c, lt * P:(lt + 1) * P],
                                wvh[:, c, :],
                                start=(c == 0), stop=(c == KO - 1),
                            )
                        nc.scalar.copy(out=Vb[:, tm, :], in_=pr[:, :E])

            # ------------- Phase D: attention, software-pipelined depth 2
            with (
                tc.tile_pool(name="psS", bufs=3, space="PSUM") as psS,
                tc.tile_pool(name="psO", bufs=2, space="PSUM") as psO,
                tc.tile_pool(name="attn_sb", bufs=3) as attn_sb,
                tc.tile_pool(name="attn_small", bufs=4) as attn_small,
            ):
                pending = []

                def issue_dots(h, it):
                    """dots for (h, it): 2 stationaries, 8 matmuls, 2 S halves."""
                    isl = slice(it * P, (it + 1) * P)
                    S0 = psS.tile([P, 1024], F32, tag="S")
                    S1 = psS.tile([P, 1024], F32, tag="S")
                    halves = [S0, S1]
                    HOIST = True
                    if HOIST:
                        for nn in range(4):
                            ns = slice(nn * 512, (nn + 1) * 512)
                            Sh = halves[nn // 2]
                            co = (nn % 2) * 512
                            nc.tensor.matmul(
                                Sh[:, co:co + 512], QTs[0:64, h, isl], KTh[0:64, h, ns],
                                start=True, stop=False,
                            )
                        for nn in range(4):
                            ns = slice(nn * 512, (nn + 1) * 512)
                            Sh = halves[nn // 2]
                            co = (nn % 2) * 512
                            nc.tensor.matmul(
                                Sh[:, co:co + 512], QTs[:, h, isl], KTsw[:, h, ns],
                                start=False, stop=True,
                            )
                    else:
                        for nn in range(4):
                            ns = slice(nn * 512, (nn + 1) * 512)
                            Sh = halves[nn // 2]
                            co = (nn % 2) * 512
                            nc.tensor.matmul(
                                Sh[:, co:co + 512], QTs[0:64, h, isl], KTh[0:64, h, ns],
                                start=True, stop=False,
                            )
                            nc.tensor.matmul(
                                Sh[:, co:co + 512], QTs[:, h, isl], KTsw[:, h, ns],
                                start=False, stop=True,
                            )
                    return halves

                def issue_softmax(h, it, halves):
                    mx2 = attn_small.tile([P, 2], F32, tag="mx2")
                    for half in range(2):
                        nc.vector.tensor_reduce(
                            out=mx2[:, half:half + 1], in_=halves[half],
                            axis=AX, op=MAX,
                        )
                    nmx = attn_small.tile([P, 1], F32, tag="nmx")
                    nc.vector.tensor_reduce(
                        out=nmx, in_=mx2, axis=AX, op=MAX, negate=True
                    )
                    bias8 = attn_small.tile([P, 1], F32, tag="bias8")
                    nc.vector.tensor_scalar_mul(bias8, nmx, SCALE)
                    den2 = attn_small.tile([P, 2], F32, tag="den2")
                    Pb = attn_sb.tile([P, NTOK], BF16, tag="Pb")
                    for half in range(2):
                        nc.scalar.activation(
                            out=Pb[:, half * 1024:(half + 1) * 1024],
                            in_=halves[half], func=EXP,
                            bias=bias8, scale=SCALE,
                            accum_out=den2[:, half:half + 1],
                        )
                    PT = attn_sb.tile([P, TT, P], BF16, tag="PT")
                    nc.sync.dma_start_transpose(out=PT, in_=Pb)
                    return PT, den2

                def issue_pv(h, it, PT, den2):
                    # den/recip issued here (2 iterations after the exp) so the
                    # in-order DVE never stalls waiting on ACT
                    den = attn_small.tile([P, 1], F32, tag="den")
                    nc.vector.tensor_reduce(out=den, in_=den2, axis=AX, op=ADD)
                    rec = attn_small.tile([P, 1], F32, tag="rec")
                    nc.vector.reciprocal(out=rec, in_=den)
                    O_ps = psO.tile([P, D], F32, tag="O")
                    for jo in range(TT):
                        nc.tensor.matmul(
                            O_ps[:, :], PT[:, jo, :], Vb[:, jo, h * D:(h + 1) * D],
                            start=(jo == 0), stop=(jo == TT - 1),
                        )
                    nc.vector.tensor_scalar_mul(
                        Ob[:, it, h * D:(h + 1) * D], O_ps[:, :], rec
                    )

                PIPELINE = True
                for h in range(NH):
                    for it in range(TT):
                        halves = issue_dots(h, it)
                        if PIPELINE and len(pending) >= 2:
                            issue_pv(*pending.pop(0))
                        PT, den2 = issue_softmax(h, it, halves)
                        pending.append((h, it, PT, den2))
                        if not PIPELINE:
                            issue_pv(*pending.pop(0))
                while pending:
                    issue_pv(*pending.pop(0))

            # ------------- Phase E: O^T then y = O @ wo
            with (
                tc.tile_pool(name="psE", bufs=2, space="PSUM") as psE,
                tc.tile_pool(name="ysb", bufs=3) as ysb,
            ):
                for tt in range(TT):
                    ts = slice(tt * P, (tt + 1) * P)
                    for eo in range(2):
                        pt = psE.tile([P, P], BF16, tag="pt")
                        nc.tensor.transpose(
                            pt[:, :], Ob[:, tt, eo * P:(eo + 1) * P], ident[:, :]
                        )
                        nc.vector.tensor_copy(out=OTb[:, eo, ts], in_=pt)
                for tm in range(TT):
                    ms = slice(tm * P, (tm + 1) * P)
                    for n in range(2):
                        ns = slice(n * 512, (n + 1) * 512)
                        yp = psE.tile([P, 512], F32, tag="yp")
                        for eo in range(2):
                            nc.tensor.matmul(
                                yp[:, :], OTb[:, eo, ms], wob[:, eo, ns],
                                start=(eo == 0), stop=(eo == 1),
                            )
                        yo = ysb.tile([P, 512], F32, tag="yo")
                        nc.scalar.copy(out=yo, in_=yp)
                        nc.sync.dma_start(out=y[ms, ns], in_=yo)

    nc.compile()
    return nc


_NC_CACHE = None


def _get_nc():
    global _NC_CACHE
    if _NC_CACHE is None:
        _NC_CACHE = build_attention_nc()
    return _NC_CACHE


def kernel(x, w_q, w_vk, w_out, **run_kwargs):
    """Full inputs in, full output out. Shards over 8 NeuronCores."""
    b, n, dim = x.shape
    assert (b, n, dim) == (2, 2048, 1024)
    w_k = w_vk[:, :1024]
    w_v = w_vk[:, 1024:]

    in_maps = []
    for c in range(8):
        bi = c // 4
        hg = c % 4
        cs = slice(hg * E, (hg + 1) * E)
        in_maps.append({
            "x": np.ascontiguousarray(x[bi]).astype(np.float32),
            "wq": np.ascontiguousarray(w_q[:, cs]).astype(np.float32),
            "wk": np.ascontiguousarray(w_k[:, cs]).astype(np.float32),
            "wv": np.ascontiguousarray(w_v[:, cs]).astype(np.float32),
            "wo": np.ascontiguousarray(w_out[cs, :]).astype(np.float32),
        })

    nc = _get_nc()
    res = run_bass_kernel_spmd(nc, in_maps, core_ids=list(range(8)), **run_kwargs)
    out = np.zeros((2, 2048, 1024), dtype=np.float32)
    for c in range(8):
        out[c // 4] += res.results[c]["y"]
    if run_kwargs:
        kernel.last_results = res
    return out


# revision 17
# speedup vs baseline: 1.2339x; 1.1292x over previous
"""Trainium2 Bass kernel for nn_Attention_49907519980190.

Reference computation (b=2, n=2048, dim=1024, h=16, d=64):
    q = (x @ w_q)   -> (b, h, n, d)
    k, v = split(x @ w_vk)
    dots = (q @ k^T) * sqrt(d)          # NOTE: multiplies by 8
    attn = softmax(dots)
    out = (attn @ v) reassembled -> (b, n, h*d) @ w_out

Sharding (8 cores): batch x head-group parallel. Core c handles batch
b = c // 4 and heads 4*(c % 4) .. 4*(c % 4) + 4. Column-parallel
q/k/v projections, row-parallel out projection; the host sums the four
partial outputs per batch (the "all-reduce" of row-parallel TP).

Numerics: the softmax logits have std ~75 and the softmax is ~97%
one-hot, so Q/K/dots need fp32-class precision. The PE's in-matmul
accumulator is block-aligned (drops addends ~2^-11 below the running
max) but PSUM accumulation BETWEEN matmuls is exact fp32. So Q, K and
dots use a bf16 hi/lo decomposition: x = hi + lo with both bf16;
a@b ~= ah@bh + (al@bh + ah@bl) as separate accumulating matmuls
(measured 4.5e-6 matmul rel err vs 2.4e-3 plain bf16). V, attn@V and
the out projection are plain bf16 (errors there average out).

Schedule notes (from perfetto): keep the PE stream free of stalls --
software-pipeline the attention inner loop (PV for iteration N-2 is
issued between the dots of iteration N), start projections as soon as
the first token-group of x^T lands (x^T is split into 4 token-group
tiles), and spread DMA issue / PSUM eviction across sync, scalar,
vector and gpsimd so no engine's sequencer serializes the chain.
"""

import numpy as np

import concourse.bass as bass
import concourse.mybir as mybir
import concourse.tile as tile
from concourse import bacc
from concourse.bass_utils import run_bass_kernel_spmd
from concourse.masks import make_identity

F32 = mybir.dt.float32
BF16 = mybir.dt.bfloat16
SUB = mybir.AluOpType.subtract
MAX = mybir.AluOpType.max
ADD = mybir.AluOpType.add
AX = mybir.AxisListType.X
EXP = mybir.ActivationFunctionType.Exp

P = 128      # partitions
NTOK = 2048  # tokens per core (one batch slice)
DIM = 1024   # model dim
E = 256      # per-core projection width (4 heads x 64)
NH = 4       # heads per core
D = 64       # head dim
KO = 8       # contraction chunks of 128 over DIM
TT = 16      # token tiles of 128
NG = 4       # token groups (of 512) for x^T
SCALE = 8.0  # sqrt(D); reference MULTIPLIES by it


def build_attention_nc():
    nc = bacc.Bacc("TRN2", target_bir_lowering=False, debug=False)

    x = nc.declare_dram_parameter("x", [NTOK, DIM], F32, isOutput=False)
    wq = nc.declare_dram_parameter("wq", [DIM, E], F32, isOutput=False)
    wk = nc.declare_dram_parameter("wk", [DIM, E], F32, isOutput=False)
    wv = nc.declare_dram_parameter("wv", [DIM, E], F32, isOutput=False)
    wo = nc.declare_dram_parameter("wo", [E, DIM], F32, isOutput=False)
    y = nc.declare_dram_parameter("y", [NTOK, DIM], F32, isOutput=True)

    with tile.TileContext(nc) as tc:
        with tc.tile_pool(name="persist", bufs=1) as persist:
            # x^T hi/lo bf16 in 4 token-group tiles: [dim_low, dim_hi, 512 tok]
            xh = []
            xl = []
            for g in range(NG):
                xh_g = persist.tile([P, KO, 512], BF16, tag=f"xh{g}")
                xl_g = persist.tile([P, KO, 512], BF16, tag=f"xl{g}")
                xh.append(xh_g)
                xl.append(xl_g)
            # Q^T stacked per head: rows 0:64 = q_hi, 64:128 = q_lo
            QTs = persist.tile([P, NH, NTOK], BF16)
            # K^T swapped-stacked: rows 0:64 = k_lo, 64:128 = k_hi
            KTsw = persist.tile([P, NH, NTOK], BF16)
            # K^T hi alone (base partition 0, for the main dots matmul)
            KTh = persist.tile([64, NH, NTOK], BF16)
            # V natural [tok_low, tok_hi, emb], O natural likewise
            Vb = persist.tile([P, TT, E], BF16)
            Ob = persist.tile([P, TT, E], BF16)
            # O^T [emb_low, emb_hi(2), tok]
            OTb = persist.tile([P, 2, NTOK], BF16)
            # weights
            wqh = persist.tile([P, KO, E], BF16)
            wql = persist.tile([P, KO, E], BF16)
            wkh = persist.tile([P, KO, E], BF16)
            wkl = persist.tile([P, KO, E], BF16)
            wvh = persist.tile([P, KO, E], BF16)
            wob = persist.tile([P, 2, DIM], BF16)
            ident = persist.tile([P, P], BF16)
            make_identity(nc, ident)

            # ------------- Phase A/B: weights, then x (split hi/lo + transpose)
            with tc.tile_pool(name="stage", bufs=3) as stage:
                # weights first so projections can start the moment x^T lands
                for wsrc, hdst, ldst in ((wk, wkh, wkl), (wq, wqh, wql)):
                    wf = stage.tile([P, KO, E], F32, tag="wf", bufs=1)
                    nc.sync.dma_start(
                        out=wf, in_=wsrc[:, :].rearrange("(ko p) e -> p ko e", p=P)
                    )
                    nc.vector.tensor_copy(out=hdst, in_=wf)
                    nc.vector.tensor_tensor(out=ldst, in0=wf, in1=hdst, op=SUB)
                wf = stage.tile([P, KO, E], F32, tag="wf", bufs=1)
                nc.sync.dma_start(
                    out=wf, in_=wv[:, :].rearrange("(ko p) e -> p ko e", p=P)
                )
                nc.vector.tensor_copy(out=wvh, in_=wf)
                wof = stage.tile([P, 2, DIM], F32, tag="wof", bufs=1)
                nc.sync.dma_start(
                    out=wof, in_=wo[:, :].rearrange("(eo p) d -> p eo d", p=P)
                )
                nc.vector.tensor_copy(out=wob, in_=wof)

                for tt in range(TT):
                    g, lt = tt // 4, tt % 4
                    ts = slice(tt * P, (tt + 1) * P)
                    gs = slice(lt * P, (lt + 1) * P)
                    xf = stage.tile([P, DIM], F32, tag="xf")
                    nc.gpsimd.dma_start(out=xf, in_=x[ts, :])
                    xhn = stage.tile([P, DIM], BF16, tag="xhn")
                    xln = stage.tile([P, DIM], BF16, tag="xln")
                    nc.vector.tensor_copy(out=xhn, in_=xf)
                    nc.vector.tensor_tensor(out=xln, in0=xf, in1=xhn, op=SUB)
                    nc.sync.dma_start_transpose(out=xh[g][:, :, gs], in_=xhn)
                    nc.sync.dma_start_transpose(out=xl[g][:, :, gs], in_=xln)

            # ------------- Phase C: projections K^T, Q^T (hi/lo), V (plain)
            with tc.tile_pool(name="psA", bufs=2, space="PSUM") as psA:
                def proj_hilo(wh, wl, m, g):
                    pr = psA.tile([P, 512], F32, tag="pr")
                    ms = slice(m * P, (m + 1) * P)
                    for c in range(KO):
                        nc.tensor.matmul(
                            pr[:, :], wh[:, c, ms], xh[g][:, c, :],
                            start=(c == 0), stop=False,
                        )
                    for c in range(KO):
                        nc.tensor.matmul(
                            pr[:, :], wl[:, c, ms], xh[g][:, c, :],
                            start=False, stop=False,
                        )
                    for c in range(KO):
                        nc.tensor.matmul(
                            pr[:, :], wh[:, c, ms], xl[g][:, c, :],
                            start=False, stop=(c == KO - 1),
                        )
                    return pr

                for g in range(NG):
                    ns = slice(g * 512, (g + 1) * 512)
                    for m in range(2):
                        pr = proj_hilo(wkh, wkl, m, g)
                        for hh in range(2):
                            h = 2 * m + hh
                            rows = slice(hh * 64, hh * 64 + 64)
                            nc.scalar.copy(out=KTh[0:64, h, ns], in_=pr[rows, :])
                            nc.scalar.copy(out=KTsw[64:128, h, ns], in_=pr[rows, :])
                            nc.vector.tensor_tensor(
                                out=KTsw[0:64, h, ns], in0=pr[rows, :],
                                in1=KTh[0:64, h, ns], op=SUB,
                            )
                    for m in range(2):
                        pr = proj_hilo(wqh, wql, m, g)
                        for hh in range(2):
                            h = 2 * m + hh
                            rows = slice(hh * 64, hh * 64 + 64)
                            nc.scalar.copy(out=QTs[0:64, h, ns], in_=pr[rows, :])
                            nc.vector.tensor_tensor(
                                out=QTs[64:128, h, ns], in0=pr[rows, :],
                                in1=QTs[0:64, h, ns], op=SUB,
                            )
                    for lt in range(4):
                        tm = g * 4 + lt
                        pr = psA.tile([P, 512], F32, tag="pr")
                        for c in range(KO):
                            nc.tensor.matmul(
                                pr[:, :E], xh[g][:, c, lt * P:(lt + 1) * P],
                                wvh[:, c, :],
                                start=(c == 0), stop=(c == KO - 1),
                            )
                        nc.scalar.copy(out=Vb[:, tm, :], in_=pr[:, :E])

            # ------------- Phase D: attention, software-pipelined depth 2
            with (
                tc.tile_pool(name="psS", bufs=6, space="PSUM") as psS,
                tc.tile_pool(name="psO", bufs=2, space="PSUM") as psO,
                tc.tile_pool(name="attn_sb", bufs=4) as attn_sb,
                tc.tile_pool(name="attn_small", bufs=6) as attn_small,
            ):
                pending = []

                def issue_dots(h, it):
                    """dots for (h, it): 2 stationaries, 8 matmuls, 4 S quarters."""
                    isl = slice(it * P, (it + 1) * P)
                    quarters = []
                    for nn in range(4):
                        Sq = psS.tile([P, 512], F32, tag="S")
                        quarters.append(Sq)
                    # main pass: stationary q_hi (one LDW), 4 chunks
                    for nn in range(4):
                        ns = slice(nn * 512, (nn + 1) * 512)
                        nc.tensor.matmul(
                            quarters[nn][:, :], QTs[0:64, h, isl], KTh[0:64, h, ns],
                            start=True, stop=False,
                        )
                    # correction pass: stationary [q_hi; q_lo] (one LDW)
                    for nn in range(4):
                        ns = slice(nn * 512, (nn + 1) * 512)
                        nc.tensor.matmul(
                            quarters[nn][:, :], QTs[:, h, isl], KTsw[:, h, ns],
                            start=False, stop=True,
                        )
                    return quarters

                def issue_softmax(h, it, quarters):
                    mx4 = attn_small.tile([P, 4], F32, tag="mx4")
                    for nn in range(4):
                        nc.vector.tensor_reduce(
                            out=mx4[:, nn:nn + 1], in_=quarters[nn], axis=AX, op=MAX
                        )
                    nmx = attn_small.tile([P, 1], F32, tag="nmx")
                    nc.vector.tensor_reduce(
                        out=nmx, in_=mx4, axis=AX, op=MAX, negate=True
                    )
                    bias8 = attn_small.tile([P, 1], F32, tag="bias8")
                    nc.vector.tensor_scalar_mul(bias8, nmx, SCALE)
                    den4 = attn_small.tile([P, 4], F32, tag="den4")
                    Pb = attn_sb.tile([P, NTOK], BF16, tag="Pb")
                    for nn in range(4):
                        nc.scalar.activation(
                            out=Pb[:, nn * 512:(nn + 1) * 512],
                            in_=quarters[nn], func=EXP,
                            bias=bias8, scale=SCALE,
                            accum_out=den4[:, nn:nn + 1],
                        )
                    PT = attn_sb.tile([P, TT, P], BF16, tag="PT")
                    nc.sync.dma_start_transpose(out=PT, in_=Pb)
                    return PT, den4

                def issue_pv(h, it, PT, den4):
                    # den/recip issued here (N iterations after the exp) so the
                    # in-order DVE never stalls waiting on ACT
                    den = attn_small.tile([P, 1], F32, tag="den")
                    nc.vector.tensor_reduce(out=den, in_=den4, axis=AX, op=ADD)
                    rec = attn_small.tile([P, 1], F32, tag="rec")
                    nc.vector.reciprocal(out=rec, in_=den)
                    O_ps = psO.tile([P, D], F32, tag="O")
                    for jo in range(TT):
                        nc.tensor.matmul(
                            O_ps[:, :], PT[:, jo, :], Vb[:, jo, h * D:(h + 1) * D],
                            start=(jo == 0), stop=(jo == TT - 1),
                        )
                    nc.vector.tensor_scalar_mul(
                        Ob[:, it, h * D:(h + 1) * D], O_ps[:, :], rec
                    )

                DEPTH = 3
                for h in range(NH):
                    for it in range(TT):
                        quarters = issue_dots(h, it)
                        if len(pending) >= DEPTH:
                            issue_pv(*pending.pop(0))
                        PT, den4 = issue_softmax(h, it, quarters)
                        pending.append((h, it, PT, den4))
                while pending:
                    issue_pv(*pending.pop(0))

            # ------------- Phase E: O^T then y = O @ wo
            with (
                tc.tile_pool(name="psE", bufs=2, space="PSUM") as psE,
                tc.tile_pool(name="ysb", bufs=3) as ysb,
            ):
                for tt in range(TT):
                    ts = slice(tt * P, (tt + 1) * P)
                    for eo in range(2):
                        pt = psE.tile([P, P], BF16, tag="pt")
                        nc.tensor.transpose(
                            pt[:, :], Ob[:, tt, eo * P:(eo + 1) * P], ident[:, :]
                        )
                        nc.vector.tensor_copy(out=OTb[:, eo, ts], in_=pt)
                for tm in range(TT):
                    ms = slice(tm * P, (tm + 1) * P)
                    for n in range(2):
                        ns = slice(n * 512, (n + 1) * 512)
                        yp = psE.tile([P, 512], F32, tag="yp")
                        for eo in range(2):
                            nc.tensor.matmul(
                                yp[:, :], OTb[:, eo, ms], wob[:, eo, ns],
                                start=(eo == 0), stop=(eo == 1),
                            )
                        yo = ysb.tile([P, 512], F32, tag="yo")
                        nc.scalar.copy(out=yo, in_=yp)
                        nc.sync.dma_start(out=y[ms, ns], in_=yo)

    nc.compile()
    return nc


_NC_CACHE = None


def _get_nc():
    global _NC_CACHE
    if _NC_CACHE is None:
        _NC_CACHE = build_attention_nc()
    return _NC_CACHE


def kernel(x, w_q, w_vk, w_out, **run_kwargs):
    """Full inputs in, full output out. Shards over 8 NeuronCores."""
    b, n, dim = x.shape
    assert (b, n, dim) == (2, 2048, 1024)
    w_k = w_vk[:, :1024]
    w_v = w_vk[:, 1024:]

    in_maps = []
    for c in range(8):
        bi = c // 4
        hg = c % 4
        cs = slice(hg * E, (hg + 1) * E)
        in_maps.append({
            "x": np.ascontiguousarray(x[bi]).astype(np.float32),
            "wq": np.ascontiguousarray(w_q[:, cs]).astype(np.float32),
            "wk": np.ascontiguousarray(w_k[:, cs]).astype(np.float32),
            "wv": np.ascontiguousarray(w_v[:, cs]).astype(np.float32),
            "wo": np.ascontiguousarray(w_out[cs, :]).astype(np.float32),
        })

    nc = _get_nc()
    res = run_bass_kernel_spmd(nc, in_maps, core_ids=list(range(8)), **run_kwargs)
    out = np.zeros((2, 2048, 1024), dtype=np.float32)
    for c in range(8):
        out[c // 4] += res.results[c]["y"]
    if run_kwargs:
        kernel.last_results = res
    return out
